# revision 7
# baseline (speedup 1.0000x reference)
"""Trainium2 Bass kernel for nn_AvgPoolVectorsPerWSI (segment-mean over groups).

Math: x [N=2048, M=512, 7, 7], idx [N] in [0,64)
  out[g, m] = mean over {n: idx[n]==g} and spatial of x[n, m, :, :]  -> [64, 512, 1, 1]

Strategy (no collectives needed):
  - Shard over M: core k handles an m-slice of 64 channels, reading its x
    slice [2048, 64, 49] (25.7 MB) exactly once -> memory-bound.
  - The HBM stream is executed by 16 SDMA engines at ~469 ns per 12.5 KB
    row-descriptor (~26.7 GB/s each, the per-engine cap).  A transfer of
    R rows is split over k = (largest divisor of R that is <= 16)
    engines, engines 0..k-1, each taking R/k CONSECUTIVE rows (measured
    from HWDGE traces; e.g. R=92 uses only 4 engines at 23 rows each).
    Engine 15 is empirically ~20-25% slower in many periods, and with a
    uniform 16x128-row schedule it alone sets the stream time (76 us vs
    60 us).
  - Rebalanced schedule: 17 rounds sized 8x128 + 4x120 + 4x112 + 1x96
    (= 2048 rows).  120-row transfers use 15 engines (engine 15 idle);
    112/96-row transfers use 16 engines at 7/6 rows.  Row totals per
    engine: e15 = 98, e0-14 = 130 -> stream ~61 us whether or not
    engine 15 is degraded.  Inactive (round, partition) slots get
    weight 0 (host writes idx=-1 -> is_equal misses) and the PE reads
    stale-but-finite data from 8 rounds earlier (the 8 full rounds run
    first, so every buffer slot is fully written before any partial
    round reuses it -> no uninitialized SBUF is ever read).
  - All compute is fp32-exact.  Per round the work is split so both
    engines stay under the ~3.4-3.8 us/round DMA pace:
      * TensorE, m-channels [0, MC): fused segment-sum on raw x,
          psum_big[g, (m,j)] += w[n, g]^T @ x[n, (m,j)]
      * VectorE, m-channels [MC, 64): spatial j-reduce to xs[n, m], then
        a small fp32 matmul accumulates psum_small[g, m] += w^T @ xs
    with w the scale-weighted one-hot (scale = 1/(count_g*49)), generated
    on device from a small aux tensor (iota/scale/per-round idx).
  - Epilogue (pipelined with the last round's matmul chunks): j-reduce
    psum_big -> out[:, :MC] in three m-chunks, copy psum_small ->
    out[:, MC:], and DMA out in four pieces as each chunk finishes.
    Host concatenates the 8 core results along m.

Raw Block implementation (not Tile): the walrus matmul/DMA lowerings only
accept ONE attached sync-wait per instruction; standalone wait_ge
instructions sidestep that.

DMA-completion semaphores: round r uses sem r % BUFS with a cumulative
threshold (16 per dma_start piece).  A shared counter is only safe
because a round's sem is reused (r+BUFS) strictly after round r was
consumed (the slot-reuse wait orders the re-issue).
"""

from contextlib import ExitStack

import numpy as np

import concourse.bass as bass
import concourse.mybir as mybir
from concourse.bass_utils import run_bass_kernel_spmd

N = 2048          # samples
M = 512           # channels
HW = 49           # spatial (7*7)
G = 64            # groups
CORES = 8
ML = M // CORES   # 64 channels per core
F = ML * HW       # 3136 floats per (n, core)
P = 128           # partitions per full tile
NT = 17           # rounds
BUFS = 8          # x-tile buffer depth == number of DMA semaphores

MC = 26           # m-channels handled by TensorE (raw fused matmul)
MV = ML - MC      # m-channels handled by VectorE reduce (38)
FC = MC * HW      # 1274 raw columns through the PE
# fp32 matmul chunks must stay within one 2KB PSUM bank -> 512-col chunks
CHUNKS = [(c * 512, min((c + 1) * 512, FC)) for c in range((FC + 511) // 512)]
NCH = len(CHUNKS)

# Rows per round: the row count alone steers the SDMA engine fan-out
# (largest divisor <= 16).  The 8 full rounds run first so every buffer
# slot is initialized before partial rounds reuse them; the smallest
# round runs last to shorten the tail.
ROWS = [128] * 8 + [120] * 4 + [112] * 4 + [96]
assert len(ROWS) == NT and sum(ROWS) == N

# cumulative dma-sem threshold for round r (sem r % BUFS, +16 per round)
CUM = [16 * (r // BUFS + 1) for r in range(NT)]

# epilogue sub-reduce m-chunks of psum_big, aligned to the matmul chunks:
# sub-chunk i needs matmul chunks 0..need_i of the last round (pe_big
# counts one inc per chunk per round, in chunk order).
SUBRED = []
for _mlo, _mhi in ((0, 10), (10, 20), (20, MC)):
    _need = next(i for i, (lo, hi) in enumerate(CHUNKS) if hi >= _mhi * HW)
    SUBRED.append((_mlo, _mhi, (NT - 1) * NCH + _need + 1))
# output DMA pieces: 3 PE m-chunks + the vector-path tail, each issued as
# soon as its fin_sem increment lands.
OPIECES = [(0, 10), (10, 20), (20, MC), (MC, ML)]

F32 = mybir.dt.float32


def _build():
    nc = bass.Bass(trn_type="TRN2", target_bir_lowering=False)
    x_ext = nc.declare_dram_parameter("x", [N, F], F32, isOutput=False)
    # aux[:, 0:64] iota row, aux[:, 64:128] scale row, aux[:, 128:145] idx
    aux_ext = nc.declare_dram_parameter("aux", [P, G + G + NT], F32,
                                        isOutput=False)
    out_ext = nc.declare_dram_parameter("out", [G, ML], F32, isOutput=True)

    x_flat = x_ext.ap()  # [2048, 3136], rows pre-permuted into round order

    # HBM row offset of each round's first row
    roff = np.cumsum([0] + ROWS).tolist()

    with ExitStack() as ctx:
        x_buf = ctx.enter_context(nc.sbuf_tensor([P, BUFS * F], F32))
        xs_buf = ctx.enter_context(nc.sbuf_tensor([P, BUFS * MV], F32))
        aux_sb = ctx.enter_context(nc.sbuf_tensor([P, G + G + NT], F32))
        w_sb = ctx.enter_context(nc.sbuf_tensor([P, NT * G], F32))
        out_sb = ctx.enter_context(nc.sbuf_tensor([G, ML], F32))
        psum_big = ctx.enter_context(nc.psum_tensor([G, FC], F32))
        psum_small = ctx.enter_context(nc.psum_tensor([G, MV], F32))
        dma_x = [
            ctx.enter_context(nc.semaphore(name=f"dma_x{s}"))
            for s in range(BUFS)
        ]
        dma_a = ctx.enter_context(nc.semaphore())   # +16 when aux resident
        dma_o = ctx.enter_context(nc.semaphore())   # +16 per out piece
        wg_sem = ctx.enter_context(nc.semaphore())  # +1 when w generated
        red_sem = ctx.enter_context(nc.semaphore())  # +1 per round j-reduce
        pe_big = ctx.enter_context(nc.semaphore())   # +1 per big matmul chunk
        pe_tile = ctx.enter_context(nc.semaphore())  # +1 per round (small mm)
        fin_sem = ctx.enter_context(nc.semaphore())  # +1 per out_sb piece
        block = ctx.enter_context(nc.Block())

        def xwait(engine, r):
            engine.wait_ge(dma_x[r % BUFS], CUM[r])

        # ---- DMA program (SP / HWDGE, FIFO) ----
        @block.sync
        def _(sync):
            def xdma(r):
                if r >= BUFS:
                    # slot reuse: the small matmul is ordered after both the
                    # j-reduce and the big matmuls of its round
                    sync.wait_ge(pe_tile, r - BUFS + 1)
                slot = r % BUFS
                sync.dma_start(
                    out=x_buf[0:ROWS[r], slot * F:(slot + 1) * F],
                    in_=x_flat[roff[r]:roff[r] + ROWS[r], :],
                ).then_inc(dma_x[slot], 16)

            xdma(0)
            sync.dma_start(out=aux_sb[:, :], in_=aux_ext.ap()).then_inc(dma_a, 16)
            for r in range(1, NT):
                xdma(r)
            for i, (lo, hi) in enumerate(OPIECES):
                sync.wait_ge(fin_sem, i + 1)
                sync.dma_start(
                    out=out_ext.ap()[:, lo:hi], in_=out_sb[:, lo:hi]
                ).then_inc(dma_o, 16)
            sync.wait_ge(dma_o, 16 * len(OPIECES))

        # ---- VectorE: w generation, j-reduction, epilogue ----
        @block.vector
        def _(vector):
            # generate the scale-weighted one-hot from idx:
            #   w[p, r*G+g] = (idx[r, p] == g) * scale[g]   (idx=-1 -> 0)
            vector.wait_ge(dma_a, 16)
            for r in range(NT):
                wg = vector.scalar_tensor_tensor(
                    out=w_sb[:, r * G:(r + 1) * G],
                    in0=aux_sb[:, 0:G],
                    scalar=aux_sb[:, 2 * G + r:2 * G + r + 1],
                    in1=aux_sb[:, G:2 * G],
                    op0=mybir.AluOpType.is_equal,
                    op1=mybir.AluOpType.mult,
                )
            wg.then_inc(wg_sem, 1)

            for r in range(NT):
                xwait(vector, r)
                if r >= BUFS:
                    # xs slot reuse: wait until round r-BUFS consumed by PE
                    vector.wait_ge(pe_tile, r - BUFS + 1)
                slot = r % BUFS
                vector.tensor_reduce(
                    out=xs_buf[:, slot * MV:(slot + 1) * MV],
                    in_=x_buf[:, slot * F + FC:(slot + 1) * F].rearrange(
                        "p (m j) -> p m j", j=HW
                    ),
                    axis=mybir.AxisListType.X,
                    op=mybir.AluOpType.add,
                ).then_inc(red_sem, 1)

            # epilogue: j-reduce psum_big in m-chunks as the last round's
            # matmul chunks complete; copy psum_small
            for mlo, mhi, need in SUBRED:
                vector.wait_ge(pe_big, need)
                vector.tensor_reduce(
                    out=out_sb[:, mlo:mhi],
                    in_=psum_big[:, mlo * HW:mhi * HW].rearrange(
                        "p (m j) -> p m j", j=HW
                    ),
                    axis=mybir.AxisListType.X,
                    op=mybir.AluOpType.add,
                ).then_inc(fin_sem, 1)
            vector.wait_ge(pe_tile, NT)
            vector.tensor_copy(
                out_sb[:, MC:ML], psum_small[:, :]
            ).then_inc(fin_sem, 1)

        # ---- TensorE: segment-sum accumulation (fp32) ----
        @block.tensor
        def _(tensor):
            tensor.wait_ge(wg_sem, 1)
            for r in range(NT):
                xwait(tensor, r)
                slot = r % BUFS
                wt = w_sb[:, r * G:(r + 1) * G]
                for lo, hi in CHUNKS:
                    tensor.matmul(
                        out=psum_big[:, lo:hi],
                        lhsT=wt,
                        rhs=x_buf[:, slot * F + lo:slot * F + hi],
                        start=(r == 0),
                        stop=(r == NT - 1),
                    ).then_inc(pe_big, 1)
                tensor.wait_ge(red_sem, r + 1)
                tensor.matmul(
                    out=psum_small[:, :],
                    lhsT=wt,
                    rhs=xs_buf[:, slot * MV:(slot + 1) * MV],
                    start=(r == 0),
                    stop=(r == NT - 1),
                ).then_inc(pe_tile, 1)

    return nc


def _prepare(x, idx):
    x = np.asarray(x)
    if x.dtype != np.float32:
        x = x.astype(np.float32)
    idx = np.asarray(idx).astype(np.int64)
    counts = np.bincount(idx, minlength=G).astype(np.float64)
    scale = np.where(counts > 0, 1.0 / (counts * HW), 0.0).astype(np.float32)

    # samples fill (round, active-partition) slots in natural order, so x
    # rows stay unpermuted; only the per-slot idx placement encodes the
    # schedule.  Inactive slots keep idx=-1 -> w=0.
    aux = np.zeros((P, G + G + NT), np.float32)
    aux[:, 0:G] = np.arange(G, dtype=np.float32)[None, :]
    aux[:, G:2 * G] = scale[None, :]
    aux[:, 2 * G:] = -1.0
    n = 0
    for r in range(NT):
        cnt = ROWS[r]
        aux[0:cnt, 2 * G + r] = idx[n:n + cnt].astype(np.float32)
        n += cnt
    assert n == N

    xr = x.reshape(N, M, HW)
    in_maps = []
    for k in range(CORES):
        shard = np.ascontiguousarray(xr[:, k * ML:(k + 1) * ML, :]).reshape(N, F)
        in_maps.append({"x": shard, "aux": aux})
    return in_maps


def run(x, tensor_list_assignmentindices, trace=False):
    in_maps = _prepare(x, tensor_list_assignmentindices)
    nc = _build()
    res = run_bass_kernel_spmd(nc, in_maps, core_ids=list(range(CORES)), trace=trace)
    outs = [np.asarray(r["out"]) for r in res.results]
    out = np.concatenate(outs, axis=1)  # [G, M]
    return out.reshape(G, M, 1, 1).astype(np.float32), res.exec_time_ns


def kernel(**inputs):
    out, _ = run(inputs["x"], inputs["tensor_list_assignmentindices"], trace=False)
    return out


# revision 8
# speedup vs baseline: 1.4129x; 1.4129x over previous
"""Trainium2 Bass kernel for nn_AvgPoolVectorsPerWSI (segment-mean over groups).

Math: x [N=2048, M=512, 7, 7], idx [N] in [0,64)
  out[g, m] = mean over {n: idx[n]==g} and spatial of x[n, m, :, :]  -> [64, 512, 1, 1]

Strategy (no collectives needed):
  - Shard over M: core k handles an m-slice of 64 channels, reading its x
    slice [2048, 64, 49] (25.7 MB) exactly once -> memory-bound.  The 16
    SDMA engines execute the stream at ~469 ns per 12.5 KB row-descriptor
    (the ~27 GiB/s per-engine cap); 16 uniform 128-row rounds is the one
    transfer shape that keeps the DMA subsystem at full rate (odd row
    counts trip a ~50%-duty throttle window).
  - DUAL HWDGE QUEUES: even rounds issue on the SP queue, odd rounds on
    the ACT queue.  Each SDMA engine round-robins between the two rings,
    which hides the per-transfer descriptor-generation bubble that a
    single queue exposes (~0.9 us per round, ~14 us per run).
  - Column layout per row is [MV region | PE region] (MV first), split
    MC=26 channels to TensorE (fused segment-sum matmul, scale-weighted
    one-hot weights generated on device) and MV=38 channels to VectorE
    (spatial j-reduce + small matmul), keeping both engines under the
    ~3.8 us/round stream pace even in the PE's cold (K=4/8) state.
  - The final round is issued as four staggered column pieces (MV region,
    then one piece per PE psum chunk) with dedicated semaphores, so the
    tail compute overlaps the last landings and the PE stays warm; the
    epilogue j-reduces psum chunks as their last matmuls finish and the
    output is DMA'd in four pieces behind them.
  - All compute is fp32-exact.  Host concatenates the 8 per-core results
    along m.

Raw Block implementation (not Tile): the walrus matmul/DMA lowerings only
accept ONE attached sync-wait per instruction; standalone wait_ge
instructions sidestep that.

Only 128-row transfers (the one empirically clean SDMA shape): 16 uniform
rounds.  Column layout per row is [MV region | PE region] so the last
round's DVE j-reduce can start on piece A while piece B (PE region) is
still landing.  Output is DMA'd in 4 pieces as epilogue chunks finish.
Even rounds issue on the SP HWDGE queue, odd rounds on the ACT queue.
"""

from contextlib import ExitStack

import numpy as np

import concourse.bass as bass
import concourse.mybir as mybir
from concourse.bass_utils import run_bass_kernel_spmd

N = 2048
M = 512
HW = 49
G = 64
CORES = 8
ML = M // CORES
F = ML * HW
P = 128
NT = N // P
BUFS = 8

MC = 26
MV = ML - MC          # 34
DOFF = MV * HW        # 1666: PE region starts here (MV region first)
FC = MC * HW          # 1470
CHUNKS = [(c * 512, min((c + 1) * 512, FC)) for c in range((FC + 511) // 512)]
NCH = len(CHUNKS)
SUBRED = []
for _mlo, _mhi in ((0, 10), (10, 20), (20, MC)):
    _need = next(i for i, (lo, hi) in enumerate(CHUNKS) if hi >= _mhi * HW)
    SUBRED.append((_mlo, _mhi, (NT - 1) * NCH + _need + 1))
OPIECES = [(0, 10), (10, 20), (20, MC), (MC, ML)]

F32 = mybir.dt.float32

LAST = NT - 1
# round 15's pieces: MV region + one piece per PE chunk
PIECES = [(0, DOFF)] + [(DOFF + lo, DOFF + hi) for lo, hi in CHUNKS]


def _cum(r):
    """Threshold when round r's sem (slot r % BUFS) shows round r piece A."""
    return 16 * (r // BUFS + 1)


def _build():
    nc = bass.Bass(trn_type="TRN2", target_bir_lowering=False)
    x_ext = nc.declare_dram_parameter("x", [N, F], F32, isOutput=False)
    aux_ext = nc.declare_dram_parameter("aux", [P, G + G + NT], F32,
                                        isOutput=False)
    out_ext = nc.declare_dram_parameter("out", [G, ML], F32, isOutput=True)

    x_t = x_ext.ap().rearrange("(t p) f -> t p f", p=P)

    with ExitStack() as ctx:
        x_buf = ctx.enter_context(nc.sbuf_tensor([P, BUFS * F], F32))
        xs_buf = ctx.enter_context(nc.sbuf_tensor([P, BUFS * MV], F32))
        aux_sb = ctx.enter_context(nc.sbuf_tensor([P, G + G + NT], F32))
        w_sb = ctx.enter_context(nc.sbuf_tensor([P, NT * G], F32))
        out_sb = ctx.enter_context(nc.sbuf_tensor([G, ML], F32))
        psum_big = ctx.enter_context(nc.psum_tensor([G, FC], F32))
        psum_small = ctx.enter_context(nc.psum_tensor([G, MV], F32))
        dma_x = [
            ctx.enter_context(nc.semaphore(name=f"dma_x{s}"))
            for s in range(BUFS)
        ]
        dma_a = ctx.enter_context(nc.semaphore())
        dma_p = [
            ctx.enter_context(nc.semaphore(name=f"dma_p{i}"))
            for i in range(len(PIECES))
        ]  # +16 as each final-round piece lands
        dma_o = ctx.enter_context(nc.semaphore())
        wg_sem = ctx.enter_context(nc.semaphore())
        red_sem = ctx.enter_context(nc.semaphore())
        pe_big = ctx.enter_context(nc.semaphore())
        pe_tile = ctx.enter_context(nc.semaphore())
        fin_sem = ctx.enter_context(nc.semaphore())
        block = ctx.enter_context(nc.Block())

        def xdma(q, r):
            if r >= BUFS:
                q.wait_ge(pe_tile, r - BUFS + 1)
            slot = r % BUFS
            q.dma_start(
                out=x_buf[:, slot * F:(slot + 1) * F], in_=x_t[r]
            ).then_inc(dma_x[slot], 16)

        # ---- DMA program A (SP / HWDGE): even rounds, final round, out ----
        @block.sync
        def _(sync):
            xdma(sync, 0)
            sync.dma_start(out=aux_sb[:, :], in_=aux_ext.ap()).then_inc(dma_a, 16)
            for r in range(2, NT - 1, 2):
                xdma(sync, r)
            slot = LAST % BUFS
            sync.wait_ge(pe_tile, LAST - BUFS + 1)
            for i, (lo, hi) in enumerate(PIECES):
                sync.dma_start(
                    out=x_buf[:, slot * F + lo:slot * F + hi],
                    in_=x_t[LAST][:, lo:hi],
                ).then_inc(dma_p[i], 16)
            for i, (lo, hi) in enumerate(OPIECES):
                sync.wait_ge(fin_sem, i + 1)
                sync.dma_start(
                    out=out_ext.ap()[:, lo:hi], in_=out_sb[:, lo:hi]
                ).then_inc(dma_o, 16)
            sync.wait_ge(dma_o, 16 * len(OPIECES))

        # ---- DMA program B (ACT / HWDGE): odd rounds ----
        @block.scalar
        def _(scalar):
            for r in range(1, NT - 1, 2):
                xdma(scalar, r)

        # ---- VectorE: w generation, j-reduction, epilogue ----
        @block.vector
        def _(vector):
            vector.wait_ge(dma_a, 16)
            for t in range(NT):
                wg = vector.scalar_tensor_tensor(
                    out=w_sb[:, t * G:(t + 1) * G],
                    in0=aux_sb[:, 0:G],
                    scalar=aux_sb[:, 2 * G + t:2 * G + t + 1],
                    in1=aux_sb[:, G:2 * G],
                    op0=mybir.AluOpType.is_equal,
                    op1=mybir.AluOpType.mult,
                )
            wg.then_inc(wg_sem, 1)

            for r in range(NT):
                if r == LAST:
                    vector.wait_ge(dma_p[0], 16)
                else:
                    vector.wait_ge(dma_x[r % BUFS], _cum(r))
                if r >= BUFS:
                    vector.wait_ge(pe_tile, r - BUFS + 1)
                slot = r % BUFS
                vector.tensor_reduce(
                    out=xs_buf[:, slot * MV:(slot + 1) * MV],
                    in_=x_buf[:, slot * F:slot * F + DOFF].rearrange(
                        "p (m j) -> p m j", j=HW
                    ),
                    axis=mybir.AxisListType.X,
                    op=mybir.AluOpType.add,
                ).then_inc(red_sem, 1)

            for mlo, mhi, need in SUBRED:
                vector.wait_ge(pe_big, need)
                vector.tensor_reduce(
                    out=out_sb[:, mlo:mhi],
                    in_=psum_big[:, mlo * HW:mhi * HW].rearrange(
                        "p (m j) -> p m j", j=HW
                    ),
                    axis=mybir.AxisListType.X,
                    op=mybir.AluOpType.add,
                ).then_inc(fin_sem, 1)
            vector.wait_ge(pe_tile, NT)
            vector.tensor_copy(
                out_sb[:, MC:ML], psum_small[:, :]
            ).then_inc(fin_sem, 1)

        # ---- TensorE: segment-sum accumulation (fp32) ----
        @block.tensor
        def _(tensor):
            tensor.wait_ge(wg_sem, 1)
            for r in range(NT):
                if r != LAST:
                    tensor.wait_ge(dma_x[r % BUFS], _cum(r))
                slot = r % BUFS
                wt = w_sb[:, r * G:(r + 1) * G]
                for ci, (lo, hi) in enumerate(CHUNKS):
                    if r == LAST:
                        tensor.wait_ge(dma_p[ci + 1], 16)
                    tensor.matmul(
                        out=psum_big[:, lo:hi],
                        lhsT=wt,
                        rhs=x_buf[:, slot * F + DOFF + lo:slot * F + DOFF + hi],
                        start=(r == 0),
                        stop=(r == NT - 1),
                    ).then_inc(pe_big, 1)
                tensor.wait_ge(red_sem, r + 1)
                tensor.matmul(
                    out=psum_small[:, :],
                    lhsT=wt,
                    rhs=xs_buf[:, slot * MV:(slot + 1) * MV],
                    start=(r == 0),
                    stop=(r == NT - 1),
                ).then_inc(pe_tile, 1)

    return nc


def _prepare(x, idx):
    x = np.asarray(x)
    if x.dtype != np.float32:
        x = x.astype(np.float32)
    idx = np.asarray(idx).astype(np.int64)
    counts = np.bincount(idx, minlength=G).astype(np.float64)
    scale = np.where(counts > 0, 1.0 / (counts * HW), 0.0).astype(np.float32)
    aux = np.zeros((P, G + G + NT), np.float32)
    aux[:, 0:G] = np.arange(G, dtype=np.float32)[None, :]
    aux[:, G:2 * G] = scale[None, :]
    aux[:, 2 * G:] = idx.reshape(NT, P).T.astype(np.float32)
    xr = x.reshape(N, M, HW)
    in_maps = []
    for k in range(CORES):
        # MV channels (MC..63) first, then PE channels (0..MC)
        sl = xr[:, k * ML:(k + 1) * ML, :]
        shard = np.concatenate([sl[:, MC:, :], sl[:, :MC, :]], axis=1)
        shard = np.ascontiguousarray(shard).reshape(N, F)
        in_maps.append({"x": shard, "aux": aux})
    return in_maps


def run(x, tensor_list_assignmentindices, trace=False):
    in_maps = _prepare(x, tensor_list_assignmentindices)
    nc = _build()
    res = run_bass_kernel_spmd(nc, in_maps, core_ids=list(range(CORES)), trace=trace)
    outs = [np.asarray(r["out"]) for r in res.results]
    out = np.concatenate(outs, axis=1)
    return out.reshape(G, M, 1, 1).astype(np.float32), res.exec_time_ns


def kernel(**inputs):
    out, _ = run(inputs["x"], inputs["tensor_list_assignmentindices"], trace=False)
    return out


# revision 10
# speedup vs baseline: 1.4211x; 1.0058x over previous
"""Trainium2 Bass kernel for nn_AvgPoolVectorsPerWSI (segment-mean over groups).

Math: x [N=2048, M=512, 7, 7], idx [N] in [0,64)
  out[g, m] = mean over {n: idx[n]==g} and spatial of x[n, m, :, :]  -> [64, 512, 1, 1]

Strategy (no collectives needed):
  - Shard over M: core k handles an m-slice of 64 channels, reading its x
    slice [2048, 64, 49] (25.7 MB) exactly once -> memory-bound.  The 16
    SDMA engines execute the stream at ~469 ns per 12.5 KB row-descriptor
    (the ~27 GiB/s per-engine cap); 16 uniform 128-row rounds is the one
    transfer shape that keeps the DMA subsystem at full rate (odd row
    counts trip a ~50%-duty throttle window).
  - DUAL HWDGE QUEUES: even rounds issue on the SP queue, odd rounds on
    the ACT queue.  Each SDMA engine round-robins between the two rings,
    which hides the per-transfer descriptor-generation bubble that a
    single queue exposes (~0.9 us per round, ~14 us per run).
  - Column layout per row is [MV region | PE region] (MV first), split
    MC=26 channels to TensorE (fused segment-sum matmul, scale-weighted
    one-hot weights generated on device) and MV=38 channels to VectorE
    (spatial j-reduce + small matmul), keeping both engines under the
    ~3.8 us/round stream pace even in the PE's cold (K=4/8) state.
  - The final round is issued as four staggered column pieces (MV region,
    then one piece per PE psum chunk) with dedicated semaphores, so the
    tail compute overlaps the last landings and the PE stays warm; the
    epilogue j-reduces psum chunks as their last matmuls finish and the
    output is DMA'd in four pieces behind them.
  - All compute is fp32-exact.  Host concatenates the 8 per-core results
    along m.

Raw Block implementation (not Tile): the walrus matmul/DMA lowerings only
accept ONE attached sync-wait per instruction; standalone wait_ge
instructions sidestep that.
"""

from contextlib import ExitStack

import numpy as np

import concourse.bass as bass
import concourse.mybir as mybir
from concourse.bass_utils import run_bass_kernel_spmd

N = 2048
M = 512
HW = 49
G = 64
CORES = 8
ML = M // CORES
F = ML * HW
P = 128
NT = N // P
BUFS = 8

MC = 26
MV = ML - MC          # 38
DOFF = MV * HW        # 1862: PE region starts here (MV region first)
FC = MC * HW          # 1274
CHUNKS = [(c * 512, min((c + 1) * 512, FC)) for c in range((FC + 511) // 512)]
NCH = len(CHUNKS)
SUBRED = []
for _mlo, _mhi in ((0, 10), (10, 20), (20, MC)):
    _need = next(i for i, (lo, hi) in enumerate(CHUNKS) if hi >= _mhi * HW)
    SUBRED.append((_mlo, _mhi, (NT - 1) * NCH + _need + 1))
OPIECES = [(0, 10), (10, 20), (20, MC), (MC, ML)]

F32 = mybir.dt.float32

LAST = NT - 1
# round 15's pieces: MV region + one piece per PE chunk
PIECES = [(0, DOFF)] + [(DOFF + lo, DOFF + hi) for lo, hi in CHUNKS]


def _cum(r):
    """Threshold when round r's sem (slot r % BUFS) shows round r piece A."""
    return 16 * (r // BUFS + 1)


def _build():
    nc = bass.Bass(trn_type="TRN2", target_bir_lowering=False)
    x_ext = nc.declare_dram_parameter("x", [N, F], F32, isOutput=False)
    aux_ext = nc.declare_dram_parameter("aux", [P, G + G + NT], F32,
                                        isOutput=False)
    out_ext = nc.declare_dram_parameter("out", [G, ML], F32, isOutput=True)

    x_t = x_ext.ap().rearrange("(t p) f -> t p f", p=P)

    with ExitStack() as ctx:
        x_buf = ctx.enter_context(nc.sbuf_tensor([P, BUFS * F], F32))
        xs_buf = ctx.enter_context(nc.sbuf_tensor([P, BUFS * MV], F32))
        aux_sb = ctx.enter_context(nc.sbuf_tensor([P, G + G + NT], F32))
        w_sb = ctx.enter_context(nc.sbuf_tensor([P, NT * G], F32))
        out_sb = ctx.enter_context(nc.sbuf_tensor([G, ML], F32))
        psum_big = ctx.enter_context(nc.psum_tensor([G, FC], F32))
        psum_small = ctx.enter_context(nc.psum_tensor([G, MV], F32))
        dma_x = [
            ctx.enter_context(nc.semaphore(name=f"dma_x{s}"))
            for s in range(BUFS)
        ]
        dma_a = ctx.enter_context(nc.semaphore())
        dma_p = [
            ctx.enter_context(nc.semaphore(name=f"dma_p{i}"))
            for i in range(len(PIECES))
        ]  # +16 as each final-round piece lands
        dma_o = ctx.enter_context(nc.semaphore())
        wg_sem = ctx.enter_context(nc.semaphore())
        red_sem = ctx.enter_context(nc.semaphore())
        pe_big = ctx.enter_context(nc.semaphore())
        pe_tile = ctx.enter_context(nc.semaphore())
        fin_sem = ctx.enter_context(nc.semaphore())
        block = ctx.enter_context(nc.Block())

        def xdma(q, r):
            if r >= BUFS:
                q.wait_ge(pe_tile, r - BUFS + 1)
            slot = r % BUFS
            q.dma_start(
                out=x_buf[:, slot * F:(slot + 1) * F], in_=x_t[r]
            ).then_inc(dma_x[slot], 16)

        # ---- DMA program A (SP / HWDGE): even rounds, final round, out ----
        @block.sync
        def _(sync):
            xdma(sync, 0)
            sync.dma_start(out=aux_sb[:, :], in_=aux_ext.ap()).then_inc(dma_a, 16)
            for r in range(2, NT - 1, 2):
                xdma(sync, r)
            slot = LAST % BUFS
            sync.wait_ge(pe_tile, LAST - BUFS + 1)
            for i, (lo, hi) in enumerate(PIECES):
                sync.dma_start(
                    out=x_buf[:, slot * F + lo:slot * F + hi],
                    in_=x_t[LAST][:, lo:hi],
                ).then_inc(dma_p[i], 16)
            for i, (lo, hi) in enumerate(OPIECES):
                sync.wait_ge(fin_sem, i + 1)
                sync.dma_start(
                    out=out_ext.ap()[:, lo:hi], in_=out_sb[:, lo:hi]
                ).then_inc(dma_o, 16)
            sync.wait_ge(dma_o, 16 * len(OPIECES))

        # ---- DMA program B (ACT / HWDGE): odd rounds ----
        @block.scalar
        def _(scalar):
            for r in range(1, NT - 1, 2):
                xdma(scalar, r)

        # ---- VectorE: w generation, j-reduction, epilogue ----
        @block.vector
        def _(vector):
            vector.wait_ge(dma_a, 16)
            for t in range(NT):
                wg = vector.scalar_tensor_tensor(
                    out=w_sb[:, t * G:(t + 1) * G],
                    in0=aux_sb[:, 0:G],
                    scalar=aux_sb[:, 2 * G + t:2 * G + t + 1],
                    in1=aux_sb[:, G:2 * G],
                    op0=mybir.AluOpType.is_equal,
                    op1=mybir.AluOpType.mult,
                )
            wg.then_inc(wg_sem, 1)

            for r in range(NT):
                if r == LAST:
                    vector.wait_ge(dma_p[0], 16)
                else:
                    vector.wait_ge(dma_x[r % BUFS], _cum(r))
                if r >= BUFS:
                    vector.wait_ge(pe_tile, r - BUFS + 1)
                slot = r % BUFS
                vector.tensor_reduce(
                    out=xs_buf[:, slot * MV:(slot + 1) * MV],
                    in_=x_buf[:, slot * F:slot * F + DOFF].rearrange(
                        "p (m j) -> p m j", j=HW
                    ),
                    axis=mybir.AxisListType.X,
                    op=mybir.AluOpType.add,
                ).then_inc(red_sem, 1)

            for mlo, mhi, need in SUBRED:
                vector.wait_ge(pe_big, need)
                vector.tensor_reduce(
                    out=out_sb[:, mlo:mhi],
                    in_=psum_big[:, mlo * HW:mhi * HW].rearrange(
                        "p (m j) -> p m j", j=HW
                    ),
                    axis=mybir.AxisListType.X,
                    op=mybir.AluOpType.add,
                ).then_inc(fin_sem, 1)
            vector.wait_ge(pe_tile, NT)
            vector.tensor_copy(
                out_sb[:, MC:ML], psum_small[:, :]
            ).then_inc(fin_sem, 1)

        # ---- TensorE: segment-sum accumulation (fp32) ----
        @block.tensor
        def _(tensor):
            tensor.wait_ge(wg_sem, 1)
            for r in range(NT):
                if r != LAST:
                    tensor.wait_ge(dma_x[r % BUFS], _cum(r))
                slot = r % BUFS
                wt = w_sb[:, r * G:(r + 1) * G]
                for ci, (lo, hi) in enumerate(CHUNKS):
                    if r == LAST:
                        tensor.wait_ge(dma_p[ci + 1], 16)
                    tensor.matmul(
                        out=psum_big[:, lo:hi],
                        lhsT=wt,
                        rhs=x_buf[:, slot * F + DOFF + lo:slot * F + DOFF + hi],
                        start=(r == 0),
                        stop=(r == NT - 1),
                    ).then_inc(pe_big, 1)
                tensor.wait_ge(red_sem, r + 1)
                tensor.matmul(
                    out=psum_small[:, :],
                    lhsT=wt,
                    rhs=xs_buf[:, slot * MV:(slot + 1) * MV],
                    start=(r == 0),
                    stop=(r == NT - 1),
                ).then_inc(pe_tile, 1)

    return nc


def _prepare(x, idx):
    x = np.asarray(x)
    if x.dtype != np.float32:
        x = x.astype(np.float32)
    idx = np.asarray(idx).astype(np.int64)
    counts = np.bincount(idx, minlength=G).astype(np.float64)
    scale = np.where(counts > 0, 1.0 / (counts * HW), 0.0).astype(np.float32)
    aux = np.zeros((P, G + G + NT), np.float32)
    aux[:, 0:G] = np.arange(G, dtype=np.float32)[None, :]
    aux[:, G:2 * G] = scale[None, :]
    aux[:, 2 * G:] = idx.reshape(NT, P).T.astype(np.float32)
    xr = x.reshape(N, M, HW)
    in_maps = []
    for k in range(CORES):
        # MV channels (MC..63) first, then PE channels (0..MC)
        sl = xr[:, k * ML:(k + 1) * ML, :]
        shard = np.concatenate([sl[:, MC:, :], sl[:, :MC, :]], axis=1)
        shard = np.ascontiguousarray(shard).reshape(N, F)
        in_maps.append({"x": shard, "aux": aux})
    return in_maps


def run(x, tensor_list_assignmentindices, trace=False):
    in_maps = _prepare(x, tensor_list_assignmentindices)
    nc = _build()
    res = run_bass_kernel_spmd(nc, in_maps, core_ids=list(range(CORES)), trace=trace)
    outs = [np.asarray(r["out"]) for r in res.results]
    out = np.concatenate(outs, axis=1)
    return out.reshape(G, M, 1, 1).astype(np.float32), res.exec_time_ns


def kernel(**inputs):
    out, _ = run(inputs["x"], inputs["tensor_list_assignmentindices"], trace=False)
    return out


# revision 11
# speedup vs baseline: 1.4278x; 1.0047x over previous
"""Trainium2 Bass kernel for nn_AvgPoolVectorsPerWSI (segment-mean over groups).

Math: x [N=2048, M=512, 7, 7], idx [N] in [0,64)
  out[g, m] = mean over {n: idx[n]==g} and spatial of x[n, m, :, :]  -> [64, 512, 1, 1]

Strategy (no collectives needed):
  - Shard over M: core k handles an m-slice of 64 channels, reading its x
    slice [2048, 64, 49] (25.7 MB) exactly once -> memory-bound.  The 16
    SDMA engines execute the stream at ~469 ns per 12.5 KB row-descriptor
    (the ~27 GiB/s per-engine cap); 16 uniform 128-row rounds is the one
    transfer shape that keeps the DMA subsystem at full rate (odd row
    counts trip a ~50%-duty throttle window).
  - DUAL HWDGE QUEUES: even rounds issue on the SP queue, odd rounds on
    the ACT queue.  Each SDMA engine round-robins between the two rings,
    which hides the per-transfer descriptor-generation bubble that a
    single queue exposes (~0.9 us per round, ~14 us per run).
  - Column layout per row is [MV region | PE region] (MV first), split
    MC=26 channels to TensorE (fused segment-sum matmul, scale-weighted
    one-hot weights generated on device) and MV=38 channels to VectorE
    (spatial j-reduce + small matmul), keeping both engines under the
    ~3.8 us/round stream pace even in the PE's cold (K=4/8) state.
  - The final round is issued as five staggered column pieces (two MV
    halves, then one piece per PE psum chunk) with dedicated semaphores,
    so the tail compute overlaps the last landings and the PE stays warm;
    the epilogue j-reduces psum chunks as their last matmuls finish and
    the output is DMA'd in two pieces behind them.
  - All compute is fp32-exact.  Host concatenates the 8 per-core results
    along m.

Raw Block implementation (not Tile): the walrus matmul/DMA lowerings only
accept ONE attached sync-wait per instruction; standalone wait_ge
instructions sidestep that.
"""

from contextlib import ExitStack

import numpy as np

import concourse.bass as bass
import concourse.mybir as mybir
from concourse.bass_utils import run_bass_kernel_spmd

N = 2048
M = 512
HW = 49
G = 64
CORES = 8
ML = M // CORES
F = ML * HW
P = 128
NT = N // P
BUFS = 8

MC = 26
MV = ML - MC          # 38
DOFF = MV * HW        # 1862: PE region starts here (MV region first)
FC = MC * HW          # 1274
CHUNKS = [(c * 512, min((c + 1) * 512, FC)) for c in range((FC + 511) // 512)]
NCH = len(CHUNKS)
SUBRED = []
for _mlo, _mhi in ((0, 10), (10, 20), (20, MC)):
    _need = next(i for i, (lo, hi) in enumerate(CHUNKS) if hi >= _mhi * HW)
    SUBRED.append((_mlo, _mhi, (NT - 1) * NCH + _need + 1))
# output pieces: (col range, fin_sem threshold) — fins are S1,S2,S3,copy
OPIECES = [((0, 10), 1), ((10, ML), 4)]

F32 = mybir.dt.float32

LAST = NT - 1
# round 15's pieces: MV region in two halves + one piece per PE chunk
MVH = (MV // 2) * HW  # 931: first 19 channels
PIECES = [(0, MVH), (MVH, DOFF)] + [(DOFF + lo, DOFF + hi) for lo, hi in CHUNKS]


def _cum(r):
    """Threshold when round r's sem (slot r % BUFS) shows round r piece A."""
    return 16 * (r // BUFS + 1)


def _build():
    nc = bass.Bass(trn_type="TRN2", target_bir_lowering=False)
    x_ext = nc.declare_dram_parameter("x", [N, F], F32, isOutput=False)
    aux_ext = nc.declare_dram_parameter("aux", [P, G + G + NT], F32,
                                        isOutput=False)
    out_ext = nc.declare_dram_parameter("out", [G, ML], F32, isOutput=True)

    x_t = x_ext.ap().rearrange("(t p) f -> t p f", p=P)

    with ExitStack() as ctx:
        x_buf = ctx.enter_context(nc.sbuf_tensor([P, BUFS * F], F32))
        xs_buf = ctx.enter_context(nc.sbuf_tensor([P, BUFS * MV], F32))
        aux_sb = ctx.enter_context(nc.sbuf_tensor([P, G + G + NT], F32))
        w_sb = ctx.enter_context(nc.sbuf_tensor([P, NT * G], F32))
        out_sb = ctx.enter_context(nc.sbuf_tensor([G, ML], F32))
        psum_big = ctx.enter_context(nc.psum_tensor([G, FC], F32))
        psum_small = ctx.enter_context(nc.psum_tensor([G, MV], F32))
        dma_x = [
            ctx.enter_context(nc.semaphore(name=f"dma_x{s}"))
            for s in range(BUFS)
        ]
        dma_a = ctx.enter_context(nc.semaphore())
        dma_p = [
            ctx.enter_context(nc.semaphore(name=f"dma_p{i}"))
            for i in range(len(PIECES))
        ]  # +16 as each final-round piece lands
        dma_o = ctx.enter_context(nc.semaphore())
        wg_sem = ctx.enter_context(nc.semaphore())
        red_sem = ctx.enter_context(nc.semaphore())
        pe_big = ctx.enter_context(nc.semaphore())
        pe_tile = ctx.enter_context(nc.semaphore())
        fin_sem = ctx.enter_context(nc.semaphore())
        block = ctx.enter_context(nc.Block())

        def xdma(q, r):
            if r >= BUFS:
                q.wait_ge(pe_tile, r - BUFS + 1)
            slot = r % BUFS
            q.dma_start(
                out=x_buf[:, slot * F:(slot + 1) * F], in_=x_t[r]
            ).then_inc(dma_x[slot], 16)

        # ---- DMA program A (SP / HWDGE): even rounds, final round, out ----
        @block.sync
        def _(sync):
            xdma(sync, 0)
            sync.dma_start(out=aux_sb[:, :], in_=aux_ext.ap()).then_inc(dma_a, 16)
            for r in range(2, NT - 1, 2):
                xdma(sync, r)
            slot = LAST % BUFS
            sync.wait_ge(pe_tile, LAST - BUFS + 1)
            for i, (lo, hi) in enumerate(PIECES):
                sync.dma_start(
                    out=x_buf[:, slot * F + lo:slot * F + hi],
                    in_=x_t[LAST][:, lo:hi],
                ).then_inc(dma_p[i], 16)
            for (lo, hi), need in OPIECES:
                sync.wait_ge(fin_sem, need)
                sync.dma_start(
                    out=out_ext.ap()[:, lo:hi], in_=out_sb[:, lo:hi]
                ).then_inc(dma_o, 16)
            sync.wait_ge(dma_o, 16 * len(OPIECES))

        # ---- DMA program B (ACT / HWDGE): odd rounds ----
        @block.scalar
        def _(scalar):
            for r in range(1, NT - 1, 2):
                xdma(scalar, r)

        # ---- VectorE: w generation, j-reduction, epilogue ----
        @block.vector
        def _(vector):
            vector.wait_ge(dma_a, 16)
            for t in range(NT):
                wg = vector.scalar_tensor_tensor(
                    out=w_sb[:, t * G:(t + 1) * G],
                    in0=aux_sb[:, 0:G],
                    scalar=aux_sb[:, 2 * G + t:2 * G + t + 1],
                    in1=aux_sb[:, G:2 * G],
                    op0=mybir.AluOpType.is_equal,
                    op1=mybir.AluOpType.mult,
                )
            wg.then_inc(wg_sem, 1)

            for r in range(NT):
                if r != LAST:
                    vector.wait_ge(dma_x[r % BUFS], _cum(r))
                if r >= BUFS:
                    vector.wait_ge(pe_tile, r - BUFS + 1)
                slot = r % BUFS
                if r == LAST:
                    # two halves, pipelined with the MV pieces landing
                    for h, (clo, chi) in enumerate(((0, MVH), (MVH, DOFF))):
                        vector.wait_ge(dma_p[h], 16)
                        vector.tensor_reduce(
                            out=xs_buf[:, slot * MV + clo // HW:
                                       slot * MV + chi // HW],
                            in_=x_buf[:, slot * F + clo:slot * F + chi
                                      ].rearrange("p (m j) -> p m j", j=HW),
                            axis=mybir.AxisListType.X,
                            op=mybir.AluOpType.add,
                        ).then_inc(red_sem, 1)
                    continue
                vector.tensor_reduce(
                    out=xs_buf[:, slot * MV:(slot + 1) * MV],
                    in_=x_buf[:, slot * F:slot * F + DOFF].rearrange(
                        "p (m j) -> p m j", j=HW
                    ),
                    axis=mybir.AxisListType.X,
                    op=mybir.AluOpType.add,
                ).then_inc(red_sem, 1)

            for mlo, mhi, need in SUBRED:
                vector.wait_ge(pe_big, need)
                vector.tensor_reduce(
                    out=out_sb[:, mlo:mhi],
                    in_=psum_big[:, mlo * HW:mhi * HW].rearrange(
                        "p (m j) -> p m j", j=HW
                    ),
                    axis=mybir.AxisListType.X,
                    op=mybir.AluOpType.add,
                ).then_inc(fin_sem, 1)
            vector.wait_ge(pe_tile, NT)
            vector.tensor_copy(
                out_sb[:, MC:ML], psum_small[:, :]
            ).then_inc(fin_sem, 1)

        # ---- TensorE: segment-sum accumulation (fp32) ----
        @block.tensor
        def _(tensor):
            tensor.wait_ge(wg_sem, 1)
            for r in range(NT):
                if r != LAST:
                    tensor.wait_ge(dma_x[r % BUFS], _cum(r))
                slot = r % BUFS
                wt = w_sb[:, r * G:(r + 1) * G]
                for ci, (lo, hi) in enumerate(CHUNKS):
                    if r == LAST:
                        tensor.wait_ge(dma_p[ci + 2], 16)
                    tensor.matmul(
                        out=psum_big[:, lo:hi],
                        lhsT=wt,
                        rhs=x_buf[:, slot * F + DOFF + lo:slot * F + DOFF + hi],
                        start=(r == 0),
                        stop=(r == NT - 1),
                    ).then_inc(pe_big, 1)
                tensor.wait_ge(red_sem, r + 1 + (1 if r == LAST else 0))
                tensor.matmul(
                    out=psum_small[:, :],
                    lhsT=wt,
                    rhs=xs_buf[:, slot * MV:(slot + 1) * MV],
                    start=(r == 0),
                    stop=(r == NT - 1),
                ).then_inc(pe_tile, 1)

    return nc


def _prepare(x, idx):
    x = np.asarray(x)
    if x.dtype != np.float32:
        x = x.astype(np.float32)
    idx = np.asarray(idx).astype(np.int64)
    counts = np.bincount(idx, minlength=G).astype(np.float64)
    scale = np.where(counts > 0, 1.0 / (counts * HW), 0.0).astype(np.float32)
    aux = np.zeros((P, G + G + NT), np.float32)
    aux[:, 0:G] = np.arange(G, dtype=np.float32)[None, :]
    aux[:, G:2 * G] = scale[None, :]
    aux[:, 2 * G:] = idx.reshape(NT, P).T.astype(np.float32)
    xr = x.reshape(N, M, HW)
    in_maps = []
    for k in range(CORES):
        # MV channels (MC..63) first, then PE channels (0..MC)
        sl = xr[:, k * ML:(k + 1) * ML, :]
        shard = np.concatenate([sl[:, MC:, :], sl[:, :MC, :]], axis=1)
        shard = np.ascontiguousarray(shard).reshape(N, F)
        in_maps.append({"x": shard, "aux": aux})
    return in_maps


def run(x, tensor_list_assignmentindices, trace=False):
    in_maps = _prepare(x, tensor_list_assignmentindices)
    nc = _build()
    res = run_bass_kernel_spmd(nc, in_maps, core_ids=list(range(CORES)), trace=trace)
    outs = [np.asarray(r["out"]) for r in res.results]
    out = np.concatenate(outs, axis=1)
    return out.reshape(G, M, 1, 1).astype(np.float32), res.exec_time_ns


def kernel(**inputs):
    out, _ = run(inputs["x"], inputs["tensor_list_assignmentindices"], trace=False)
    return out


# revision 12
# speedup vs baseline: 1.4331x; 1.0037x over previous
"""Trainium2 Bass kernel for nn_AvgPoolVectorsPerWSI (segment-mean over groups).

Math: x [N=2048, M=512, 7, 7], idx [N] in [0,64)
  out[g, m] = mean over {n: idx[n]==g} and spatial of x[n, m, :, :]  -> [64, 512, 1, 1]

Strategy (no collectives needed):
  - Shard over M: core k handles an m-slice of 64 channels, reading its x
    slice [2048, 64, 49] (25.7 MB) exactly once -> memory-bound.  The 16
    SDMA engines execute the stream at ~469 ns per 12.5 KB row-descriptor
    (the ~27 GiB/s per-engine cap); 16 uniform 128-row rounds is the one
    transfer shape that keeps the DMA subsystem at full rate (odd row
    counts trip a ~50%-duty throttle window).
  - DUAL HWDGE QUEUES: even rounds issue on the SP queue, odd rounds on
    the ACT queue.  Each SDMA engine round-robins between the two rings,
    which hides the per-transfer descriptor-generation bubble that a
    single queue exposes (~0.9 us per round, ~14 us per run).
  - Column layout per row is [MV region | PE region] (MV first), split
    MC=26 channels to TensorE (fused segment-sum matmul, scale-weighted
    one-hot weights generated on device) and MV=38 channels to VectorE
    (spatial j-reduce + small matmul), keeping both engines under the
    ~3.8 us/round stream pace even in the PE's cold (K=4/8) state.
  - The final round is issued as five staggered column pieces (two MV
    halves, then one piece per PE psum chunk) with dedicated semaphores,
    so the tail compute overlaps the last landings and the PE stays warm;
    the epilogue j-reduces psum chunks as their last matmuls finish and
    the output is DMA'd in two pieces behind them.
  - All compute is fp32-exact.  Host concatenates the 8 per-core results
    along m.

Raw Block implementation (not Tile): the walrus matmul/DMA lowerings only
accept ONE attached sync-wait per instruction; standalone wait_ge
instructions sidestep that.
"""

from contextlib import ExitStack

import numpy as np

import concourse.bass as bass
import concourse.mybir as mybir
from concourse.bass_utils import run_bass_kernel_spmd

N = 2048
M = 512
HW = 49
G = 64
CORES = 8
ML = M // CORES
F = ML * HW
P = 128
NT = N // P
BUFS = 8

MC = 26
MV = ML - MC          # 38
DOFF = MV * HW        # 1862: PE region starts here (MV region first)
FC = MC * HW          # 1274
CHUNKS = [(c * 512, min((c + 1) * 512, FC)) for c in range((FC + 511) // 512)]
NCH = len(CHUNKS)
SUBRED = []
for _mlo, _mhi in ((0, 10), (10, 20), (20, MC)):
    _need = next(i for i, (lo, hi) in enumerate(CHUNKS) if hi >= _mhi * HW)
    SUBRED.append((_mlo, _mhi, (NT - 1) * NCH + _need + 1))
# output pieces: (col range, fin_sem threshold) — fins are S1,S2,S3,copy
OPIECES = [((0, 10), 1), ((10, ML), 4)]

F32 = mybir.dt.float32

LAST = NT - 1
# round 15's pieces, in issue (= landing) order: first PE chunk piece so
# the PE starts ~2 us before stream end, then the MV halves (they gate the
# long DVE reduce chain), then the remaining PE chunk pieces.
MVH = (MV // 2) * HW  # 931: first 19 channels
PIECES = [
    (DOFF + CHUNKS[0][0], DOFF + CHUNKS[0][1]),  # B1
    (0, MVH),                                    # MV half A
    (MVH, DOFF),                                 # MV half B
    (DOFF + CHUNKS[1][0], DOFF + CHUNKS[1][1]),  # B2
    (DOFF + CHUNKS[2][0], DOFF + CHUNKS[2][1]),  # B3
]
MV_PIECE = (1, 2)          # dma_p indices of the MV halves
CHUNK_PIECE = {0: 0, 1: 3, 2: 4}  # PE chunk ci -> dma_p index


def _cum(r):
    """Threshold when round r's sem (slot r % BUFS) shows round r piece A."""
    return 16 * (r // BUFS + 1)


def _build():
    nc = bass.Bass(trn_type="TRN2", target_bir_lowering=False)
    x_ext = nc.declare_dram_parameter("x", [N, F], F32, isOutput=False)
    aux_ext = nc.declare_dram_parameter("aux", [P, G + G + NT], F32,
                                        isOutput=False)
    out_ext = nc.declare_dram_parameter("out", [G, ML], F32, isOutput=True)

    x_t = x_ext.ap().rearrange("(t p) f -> t p f", p=P)

    with ExitStack() as ctx:
        x_buf = ctx.enter_context(nc.sbuf_tensor([P, BUFS * F], F32))
        xs_buf = ctx.enter_context(nc.sbuf_tensor([P, BUFS * MV], F32))
        aux_sb = ctx.enter_context(nc.sbuf_tensor([P, G + G + NT], F32))
        w_sb = ctx.enter_context(nc.sbuf_tensor([P, NT * G], F32))
        out_sb = ctx.enter_context(nc.sbuf_tensor([G, ML], F32))
        psum_big = ctx.enter_context(nc.psum_tensor([G, FC], F32))
        psum_small = ctx.enter_context(nc.psum_tensor([G, MV], F32))
        dma_x = [
            ctx.enter_context(nc.semaphore(name=f"dma_x{s}"))
            for s in range(BUFS)
        ]
        dma_a = ctx.enter_context(nc.semaphore())
        dma_p = [
            ctx.enter_context(nc.semaphore(name=f"dma_p{i}"))
            for i in range(len(PIECES))
        ]  # +16 as each final-round piece lands
        dma_o = ctx.enter_context(nc.semaphore())
        wg_sem = ctx.enter_context(nc.semaphore())
        red_sem = ctx.enter_context(nc.semaphore())
        pe_big = ctx.enter_context(nc.semaphore())
        pe_tile = ctx.enter_context(nc.semaphore())
        fin_sem = ctx.enter_context(nc.semaphore())
        block = ctx.enter_context(nc.Block())

        def xdma(q, r):
            if r >= BUFS:
                q.wait_ge(pe_tile, r - BUFS + 1)
            slot = r % BUFS
            q.dma_start(
                out=x_buf[:, slot * F:(slot + 1) * F], in_=x_t[r]
            ).then_inc(dma_x[slot], 16)

        # ---- DMA program A (SP / HWDGE): even rounds, final round, out ----
        @block.sync
        def _(sync):
            xdma(sync, 0)
            sync.dma_start(out=aux_sb[:, :], in_=aux_ext.ap()).then_inc(dma_a, 16)
            for r in range(2, NT - 1, 2):
                xdma(sync, r)
            slot = LAST % BUFS
            sync.wait_ge(pe_tile, LAST - BUFS + 1)
            for i, (lo, hi) in enumerate(PIECES):
                sync.dma_start(
                    out=x_buf[:, slot * F + lo:slot * F + hi],
                    in_=x_t[LAST][:, lo:hi],
                ).then_inc(dma_p[i], 16)
            for (lo, hi), need in OPIECES:
                sync.wait_ge(fin_sem, need)
                sync.dma_start(
                    out=out_ext.ap()[:, lo:hi], in_=out_sb[:, lo:hi]
                ).then_inc(dma_o, 16)
            sync.wait_ge(dma_o, 16 * len(OPIECES))

        # ---- DMA program B (ACT / HWDGE): odd rounds ----
        @block.scalar
        def _(scalar):
            for r in range(1, NT - 1, 2):
                xdma(scalar, r)

        # ---- VectorE: w generation, j-reduction, epilogue ----
        @block.vector
        def _(vector):
            vector.wait_ge(dma_a, 16)
            for t in range(NT):
                wg = vector.scalar_tensor_tensor(
                    out=w_sb[:, t * G:(t + 1) * G],
                    in0=aux_sb[:, 0:G],
                    scalar=aux_sb[:, 2 * G + t:2 * G + t + 1],
                    in1=aux_sb[:, G:2 * G],
                    op0=mybir.AluOpType.is_equal,
                    op1=mybir.AluOpType.mult,
                )
            wg.then_inc(wg_sem, 1)

            for r in range(NT):
                if r != LAST:
                    vector.wait_ge(dma_x[r % BUFS], _cum(r))
                if r >= BUFS:
                    vector.wait_ge(pe_tile, r - BUFS + 1)
                slot = r % BUFS
                if r == LAST:
                    # two halves, pipelined with the MV pieces landing
                    for h, (clo, chi) in enumerate(((0, MVH), (MVH, DOFF))):
                        vector.wait_ge(dma_p[MV_PIECE[h]], 16)
                        vector.tensor_reduce(
                            out=xs_buf[:, slot * MV + clo // HW:
                                       slot * MV + chi // HW],
                            in_=x_buf[:, slot * F + clo:slot * F + chi
                                      ].rearrange("p (m j) -> p m j", j=HW),
                            axis=mybir.AxisListType.X,
                            op=mybir.AluOpType.add,
                        ).then_inc(red_sem, 1)
                    continue
                vector.tensor_reduce(
                    out=xs_buf[:, slot * MV:(slot + 1) * MV],
                    in_=x_buf[:, slot * F:slot * F + DOFF].rearrange(
                        "p (m j) -> p m j", j=HW
                    ),
                    axis=mybir.AxisListType.X,
                    op=mybir.AluOpType.add,
                ).then_inc(red_sem, 1)

            for mlo, mhi, need in SUBRED:
                vector.wait_ge(pe_big, need)
                vector.tensor_reduce(
                    out=out_sb[:, mlo:mhi],
                    in_=psum_big[:, mlo * HW:mhi * HW].rearrange(
                        "p (m j) -> p m j", j=HW
                    ),
                    axis=mybir.AxisListType.X,
                    op=mybir.AluOpType.add,
                ).then_inc(fin_sem, 1)
            vector.wait_ge(pe_tile, NT)
            vector.tensor_copy(
                out_sb[:, MC:ML], psum_small[:, :]
            ).then_inc(fin_sem, 1)

        # ---- TensorE: segment-sum accumulation (fp32) ----
        @block.tensor
        def _(tensor):
            tensor.wait_ge(wg_sem, 1)
            for r in range(NT):
                if r != LAST:
                    tensor.wait_ge(dma_x[r % BUFS], _cum(r))
                slot = r % BUFS
                wt = w_sb[:, r * G:(r + 1) * G]
                for ci, (lo, hi) in enumerate(CHUNKS):
                    if r == LAST:
                        tensor.wait_ge(dma_p[CHUNK_PIECE[ci]], 16)
                    tensor.matmul(
                        out=psum_big[:, lo:hi],
                        lhsT=wt,
                        rhs=x_buf[:, slot * F + DOFF + lo:slot * F + DOFF + hi],
                        start=(r == 0),
                        stop=(r == NT - 1),
                    ).then_inc(pe_big, 1)
                tensor.wait_ge(red_sem, r + 1 + (1 if r == LAST else 0))
                tensor.matmul(
                    out=psum_small[:, :],
                    lhsT=wt,
                    rhs=xs_buf[:, slot * MV:(slot + 1) * MV],
                    start=(r == 0),
                    stop=(r == NT - 1),
                ).then_inc(pe_tile, 1)

    return nc


def _prepare(x, idx):
    x = np.asarray(x)
    if x.dtype != np.float32:
        x = x.astype(np.float32)
    idx = np.asarray(idx).astype(np.int64)
    counts = np.bincount(idx, minlength=G).astype(np.float64)
    scale = np.where(counts > 0, 1.0 / (counts * HW), 0.0).astype(np.float32)
    aux = np.zeros((P, G + G + NT), np.float32)
    aux[:, 0:G] = np.arange(G, dtype=np.float32)[None, :]
    aux[:, G:2 * G] = scale[None, :]
    aux[:, 2 * G:] = idx.reshape(NT, P).T.astype(np.float32)
    xr = x.reshape(N, M, HW)
    in_maps = []
    for k in range(CORES):
        # MV channels (MC..63) first, then PE channels (0..MC)
        sl = xr[:, k * ML:(k + 1) * ML, :]
        shard = np.concatenate([sl[:, MC:, :], sl[:, :MC, :]], axis=1)
        shard = np.ascontiguousarray(shard).reshape(N, F)
        in_maps.append({"x": shard, "aux": aux})
    return in_maps


def run(x, tensor_list_assignmentindices, trace=False):
    in_maps = _prepare(x, tensor_list_assignmentindices)
    nc = _build()
    res = run_bass_kernel_spmd(nc, in_maps, core_ids=list(range(CORES)), trace=trace)
    outs = [np.asarray(r["out"]) for r in res.results]
    out = np.concatenate(outs, axis=1)
    return out.reshape(G, M, 1, 1).astype(np.float32), res.exec_time_ns


def kernel(**inputs):
    out, _ = run(inputs["x"], inputs["tensor_list_assignmentindices"], trace=False)
    return out


# revision 14
# speedup vs baseline: 1.4343x; 1.0008x over previous
"""Trainium2 Bass kernel for nn_AvgPoolVectorsPerWSI (segment-mean over groups).

Math: x [N=2048, M=512, 7, 7], idx [N] in [0,64)
  out[g, m] = mean over {n: idx[n]==g} and spatial of x[n, m, :, :]  -> [64, 512, 1, 1]

Strategy (no collectives needed):
  - Shard over M: core k handles an m-slice of 64 channels, reading its x
    slice [2048, 64, 49] (25.7 MB) exactly once -> memory-bound.  The 16
    SDMA engines execute the stream at ~469 ns per 12.5 KB row-descriptor
    (the ~27 GiB/s per-engine cap); 16 uniform 128-row rounds is the one
    transfer shape that keeps the DMA subsystem at full rate (odd row
    counts trip a ~50%-duty throttle window).
  - DUAL HWDGE QUEUES: even rounds issue on the SP queue, odd rounds on
    the ACT queue.  Each SDMA engine round-robins between the two rings,
    which hides the per-transfer descriptor-generation bubble that a
    single queue exposes (~0.9 us per round, ~14 us per run).
  - Column layout per row is [MV region | PE region] (MV first), split
    MC=26 channels to TensorE (fused segment-sum matmul, scale-weighted
    one-hot weights generated on device) and MV=38 channels to VectorE
    (spatial j-reduce + small matmul), keeping both engines under the
    ~3.8 us/round stream pace even in the PE's cold (K=4/8) state.
  - The final round is issued as five staggered column pieces (first PE
    chunk, two MV halves, remaining PE chunks) with dedicated semaphores,
    so the tail compute overlaps the last landings and the PE stays warm;
    the epilogue j-reduces psum chunks as their last matmuls finish and
    the output is DMA'd in two pieces behind them.
  - All compute is fp32-exact.  Host concatenates the 8 per-core results
    along m.

Raw Block implementation (not Tile): the walrus matmul/DMA lowerings only
accept ONE attached sync-wait per instruction; standalone wait_ge
instructions sidestep that.
"""

from contextlib import ExitStack

import numpy as np

import concourse.bass as bass
import concourse.mybir as mybir
from concourse.bass_utils import run_bass_kernel_spmd

N = 2048
M = 512
HW = 49
G = 64
CORES = 8
ML = M // CORES
F = ML * HW
P = 128
NT = N // P
BUFS = 8

MC = 26
MV = ML - MC          # 38
DOFF = MV * HW        # 1862: PE region starts here (MV region first)
FC = MC * HW          # 1274
CHUNKS = [(c * 512, min((c + 1) * 512, FC)) for c in range((FC + 511) // 512)]
NCH = len(CHUNKS)
SUBRED = []
for _mlo, _mhi in ((0, 10), (10, 20), (20, MC)):
    _need = next(i for i, (lo, hi) in enumerate(CHUNKS) if hi >= _mhi * HW)
    SUBRED.append((_mlo, _mhi, (NT - 1) * NCH + _need + 1))
# output pieces: (col range, fin_sem threshold) — fins are S1,S2,S3,copy
OPIECES = [((0, 10), 1), ((10, ML), 4)]

F32 = mybir.dt.float32

LAST = NT - 1
# round 15's pieces, in issue (= landing) order: first PE chunk piece so
# the PE starts ~2 us before stream end, then the MV halves (they gate the
# long DVE reduce chain), then the remaining PE chunk pieces.
MVH = (MV // 2) * HW  # 931: first 19 channels
PIECES = [
    (DOFF + CHUNKS[0][0], DOFF + CHUNKS[0][1]),  # B1
    (0, MVH),                                    # MV half A
    (MVH, DOFF),                                 # MV half B
    (DOFF + CHUNKS[1][0], DOFF + CHUNKS[1][1]),  # B2
    (DOFF + CHUNKS[2][0], DOFF + CHUNKS[2][1]),  # B3
]
MV_PIECE = (1, 2)          # dma_p indices of the MV halves
CHUNK_PIECE = {0: 0, 1: 3, 2: 4}  # PE chunk ci -> dma_p index
PEN = NT - 2  # round 14: also split column-wise (MV piece, PE piece)


def _cum(r):
    """Threshold when round r's sem (slot r % BUFS) shows round r piece A."""
    return 16 * (r // BUFS + 1)


def _build():
    nc = bass.Bass(trn_type="TRN2", target_bir_lowering=False)
    x_ext = nc.declare_dram_parameter("x", [N, F], F32, isOutput=False)
    aux_ext = nc.declare_dram_parameter("aux", [P, G + G + NT], F32,
                                        isOutput=False)
    out_ext = nc.declare_dram_parameter("out", [G, ML], F32, isOutput=True)

    x_t = x_ext.ap().rearrange("(t p) f -> t p f", p=P)

    with ExitStack() as ctx:
        x_buf = ctx.enter_context(nc.sbuf_tensor([P, BUFS * F], F32))
        xs_buf = ctx.enter_context(nc.sbuf_tensor([P, BUFS * MV], F32))
        aux_sb = ctx.enter_context(nc.sbuf_tensor([P, G + G + NT], F32))
        w_sb = ctx.enter_context(nc.sbuf_tensor([P, NT * G], F32))
        out_sb = ctx.enter_context(nc.sbuf_tensor([G, ML], F32))
        psum_big = ctx.enter_context(nc.psum_tensor([G, FC], F32))
        psum_small = ctx.enter_context(nc.psum_tensor([G, MV], F32))
        dma_x = [
            ctx.enter_context(nc.semaphore(name=f"dma_x{s}"))
            for s in range(BUFS)
        ]
        dma_a = ctx.enter_context(nc.semaphore())
        dma_p = [
            ctx.enter_context(nc.semaphore(name=f"dma_p{i}"))
            for i in range(len(PIECES))
        ]  # +16 as each final-round piece lands
        dma_q = [
            ctx.enter_context(nc.semaphore(name=f"dma_q{i}"))
            for i in range(2)
        ]  # +16 as round 14's MV / PE piece lands
        dma_o = ctx.enter_context(nc.semaphore())
        wg_sem = ctx.enter_context(nc.semaphore())
        red_sem = ctx.enter_context(nc.semaphore())
        pe_big = ctx.enter_context(nc.semaphore())
        pe_tile = ctx.enter_context(nc.semaphore())
        fin_sem = ctx.enter_context(nc.semaphore())
        block = ctx.enter_context(nc.Block())

        def xdma(q, r):
            if r >= BUFS:
                q.wait_ge(pe_tile, r - BUFS + 1)
            slot = r % BUFS
            q.dma_start(
                out=x_buf[:, slot * F:(slot + 1) * F], in_=x_t[r]
            ).then_inc(dma_x[slot], 16)

        # ---- DMA program A (SP / HWDGE): even rounds, final round, out ----
        @block.sync
        def _(sync):
            xdma(sync, 0)
            sync.dma_start(out=aux_sb[:, :], in_=aux_ext.ap()).then_inc(dma_a, 16)
            for r in range(2, PEN, 2):
                xdma(sync, r)
            # round 14 as MV | PE pieces so its compute overlaps the landing
            qslot = PEN % BUFS
            sync.wait_ge(pe_tile, PEN - BUFS + 1)
            sync.dma_start(
                out=x_buf[:, qslot * F:qslot * F + DOFF],
                in_=x_t[PEN][:, 0:DOFF],
            ).then_inc(dma_q[0], 16)
            sync.dma_start(
                out=x_buf[:, qslot * F + DOFF:(qslot + 1) * F],
                in_=x_t[PEN][:, DOFF:F],
            ).then_inc(dma_q[1], 16)
            slot = LAST % BUFS
            sync.wait_ge(pe_tile, LAST - BUFS + 1)
            for i, (lo, hi) in enumerate(PIECES):
                sync.dma_start(
                    out=x_buf[:, slot * F + lo:slot * F + hi],
                    in_=x_t[LAST][:, lo:hi],
                ).then_inc(dma_p[i], 16)
            for (lo, hi), need in OPIECES:
                sync.wait_ge(fin_sem, need)
                sync.dma_start(
                    out=out_ext.ap()[:, lo:hi], in_=out_sb[:, lo:hi]
                ).then_inc(dma_o, 16)
            sync.wait_ge(dma_o, 16 * len(OPIECES))

        # ---- DMA program B (ACT / HWDGE): odd rounds ----
        @block.scalar
        def _(scalar):
            for r in range(1, NT - 1, 2):
                xdma(scalar, r)

        # ---- VectorE: w generation, j-reduction, epilogue ----
        @block.vector
        def _(vector):
            vector.wait_ge(dma_a, 16)
            for t in range(NT):
                wg = vector.scalar_tensor_tensor(
                    out=w_sb[:, t * G:(t + 1) * G],
                    in0=aux_sb[:, 0:G],
                    scalar=aux_sb[:, 2 * G + t:2 * G + t + 1],
                    in1=aux_sb[:, G:2 * G],
                    op0=mybir.AluOpType.is_equal,
                    op1=mybir.AluOpType.mult,
                )
            wg.then_inc(wg_sem, 1)

            for r in range(NT):
                if r == PEN:
                    vector.wait_ge(dma_q[0], 16)
                elif r != LAST:
                    vector.wait_ge(dma_x[r % BUFS], _cum(r))
                if r >= BUFS:
                    vector.wait_ge(pe_tile, r - BUFS + 1)
                slot = r % BUFS
                if r == LAST:
                    # two halves, pipelined with the MV pieces landing
                    for h, (clo, chi) in enumerate(((0, MVH), (MVH, DOFF))):
                        vector.wait_ge(dma_p[MV_PIECE[h]], 16)
                        vector.tensor_reduce(
                            out=xs_buf[:, slot * MV + clo // HW:
                                       slot * MV + chi // HW],
                            in_=x_buf[:, slot * F + clo:slot * F + chi
                                      ].rearrange("p (m j) -> p m j", j=HW),
                            axis=mybir.AxisListType.X,
                            op=mybir.AluOpType.add,
                        ).then_inc(red_sem, 1)
                    continue
                vector.tensor_reduce(
                    out=xs_buf[:, slot * MV:(slot + 1) * MV],
                    in_=x_buf[:, slot * F:slot * F + DOFF].rearrange(
                        "p (m j) -> p m j", j=HW
                    ),
                    axis=mybir.AxisListType.X,
                    op=mybir.AluOpType.add,
                ).then_inc(red_sem, 1)

            for mlo, mhi, need in SUBRED:
                vector.wait_ge(pe_big, need)
                vector.tensor_reduce(
                    out=out_sb[:, mlo:mhi],
                    in_=psum_big[:, mlo * HW:mhi * HW].rearrange(
                        "p (m j) -> p m j", j=HW
                    ),
                    axis=mybir.AxisListType.X,
                    op=mybir.AluOpType.add,
                ).then_inc(fin_sem, 1)
            vector.wait_ge(pe_tile, NT)
            vector.tensor_copy(
                out_sb[:, MC:ML], psum_small[:, :]
            ).then_inc(fin_sem, 1)

        # ---- TensorE: segment-sum accumulation (fp32) ----
        @block.tensor
        def _(tensor):
            tensor.wait_ge(wg_sem, 1)
            for r in range(NT):
                if r == PEN:
                    tensor.wait_ge(dma_q[1], 16)
                elif r != LAST:
                    tensor.wait_ge(dma_x[r % BUFS], _cum(r))
                slot = r % BUFS
                wt = w_sb[:, r * G:(r + 1) * G]
                for ci, (lo, hi) in enumerate(CHUNKS):
                    if r == LAST:
                        tensor.wait_ge(dma_p[CHUNK_PIECE[ci]], 16)
                    tensor.matmul(
                        out=psum_big[:, lo:hi],
                        lhsT=wt,
                        rhs=x_buf[:, slot * F + DOFF + lo:slot * F + DOFF + hi],
                        start=(r == 0),
                        stop=(r == NT - 1),
                    ).then_inc(pe_big, 1)
                tensor.wait_ge(red_sem, r + 1 + (1 if r == LAST else 0))
                tensor.matmul(
                    out=psum_small[:, :],
                    lhsT=wt,
                    rhs=xs_buf[:, slot * MV:(slot + 1) * MV],
                    start=(r == 0),
                    stop=(r == NT - 1),
                ).then_inc(pe_tile, 1)

    return nc


def _prepare(x, idx):
    x = np.asarray(x)
    if x.dtype != np.float32:
        x = x.astype(np.float32)
    idx = np.asarray(idx).astype(np.int64)
    counts = np.bincount(idx, minlength=G).astype(np.float64)
    scale = np.where(counts > 0, 1.0 / (counts * HW), 0.0).astype(np.float32)
    aux = np.zeros((P, G + G + NT), np.float32)
    aux[:, 0:G] = np.arange(G, dtype=np.float32)[None, :]
    aux[:, G:2 * G] = scale[None, :]
    aux[:, 2 * G:] = idx.reshape(NT, P).T.astype(np.float32)
    xr = x.reshape(N, M, HW)
    in_maps = []
    for k in range(CORES):
        # MV channels (MC..63) first, then PE channels (0..MC)
        sl = xr[:, k * ML:(k + 1) * ML, :]
        shard = np.concatenate([sl[:, MC:, :], sl[:, :MC, :]], axis=1)
        shard = np.ascontiguousarray(shard).reshape(N, F)
        in_maps.append({"x": shard, "aux": aux})
    return in_maps


def run(x, tensor_list_assignmentindices, trace=False):
    in_maps = _prepare(x, tensor_list_assignmentindices)
    nc = _build()
    res = run_bass_kernel_spmd(nc, in_maps, core_ids=list(range(CORES)), trace=trace)
    outs = [np.asarray(r["out"]) for r in res.results]
    out = np.concatenate(outs, axis=1)
    return out.reshape(G, M, 1, 1).astype(np.float32), res.exec_time_ns


def kernel(**inputs):
    out, _ = run(inputs["x"], inputs["tensor_list_assignmentindices"], trace=False)
    return out
